# revision 26
# baseline (speedup 1.0000x reference)
"""Trainium2 Bass kernel for EnhancedTransformerTextClassifier (v2).

Strategy: data-parallel over batch across 8 NeuronCores (4 sequences each).
The residual stream is feature-major (x^T: [feature partition, token]) so every
matmul contracts over the partition dim with no transposes.

v2 changes vs the 81ms/3.7ms baseline:
- All matmul operands in float16 (fp32 PSUM accumulation): halves SBUF/DMA and
  doubles DVE throughput; measured numpy rel-err 1.4e-3 vs the 2e-2 gate.
- All layer weights (QKV/O/W1/W2) resident in SBUF per layer, loaded once,
  streamed through all 4 sequences (baseline re-streamed FFN weights per seq).
- FFN is two dense matmul phases (all 16 mid chunks, then all 4 out chunks)
  instead of interleaved weight-streaming: long back-to-back PE streaks keep
  the HAM clock gate at 2.4 GHz (baseline spent 2.06ms of 3.68ms throttled).
- LN rsqrt computed as exp(-0.5*ln(var+eps)) so Exp/Ln/Identity/Copy all live
  in one ACT function-table set (no 1.3us table swaps except around Gelu).
- LN variance via DVE f16 square + ones-matmul reductions; per-chunk normalize
  is 2 DVE ops + 1 ACT affine (scale/bias APs).
- Softmax stays k-major: exp on ACT with the attention mask folded into the
  bias, denominator from an appended ones-column on V (augmented PV matmul).

Self-contained: hardcodes all shapes; only needs /opt/trn_rl_repo (system).
"""
import sys

sys.path.insert(0, '/opt/trn_rl_repo')

import numpy as np

B, S, D, H, L, FF, V = 32, 512, 512, 8, 6, 2048, 32000
HD = D // H          # 64
P = 128
NCORES = 8
SEQ = B // NCORES    # 4 sequences per core
M = SEQ * S          # 2048 tokens per core
DC = D // P          # 4 feature chunks
FFC = FF // P        # 16 ff chunks
EPS = 1e-5
SCALE = float(1.0 / np.sqrt(HD))

_CACHE = {}
GELU_IDENT = False  # debug: CoreSim lacks Gelu


def _patch_act_tables():
    """Restrict the ACT function-set chooser so Exp/Ln bind only to
    natural_log_exp_and_others (which genuinely contains both). The greedy
    per-function chooser otherwise binds Exp to exp_and_others (no ln) and
    thrashes table loads (1.28us each) on every exp<->ln transition."""
    import functools
    import concourse.hw_specs as hw_specs
    import concourse.bacc as bacc_mod
    import concourse.bass_interp as bass_interp
    import concourse.mybir as mybir
    if getattr(hw_specs, "_act_tables_patched", False):
        return
    AF = mybir.ActivationFunctionType
    orig = hw_specs.get_activation_tables

    @functools.cache
    def patched(module_arch):
        tables = dict(orig(module_arch))
        out = {}
        for name, funcs in tables.items():
            if name != "natural_log_exp_and_others":
                funcs = funcs - {AF.Exp, AF.Ln}
            out[name] = funcs
        return out

    hw_specs.get_activation_tables = patched
    bacc_mod.get_activation_tables = patched
    bass_interp.get_activation_tables = patched
    hw_specs._act_tables_patched = True


def _build(n_layers):
    import concourse.mybir as mybir
    import concourse.tile as tile
    from concourse import bacc
    from concourse.masks import make_identity
    import concourse.bass as bass

    _patch_act_tables()

    f32 = mybir.dt.float32
    f32r = mybir.dt.float32r
    f16 = mybir.dt.float16
    i32 = mybir.dt.int32
    AF = mybir.ActivationFunctionType
    OP = mybir.AluOpType

    GELU = AF.Identity if GELU_IDENT else AF.Gelu
    nc = bacc.Bacc("TRN2", target_bir_lowering=False, debug=False)

    # ---------------- DRAM parameters ----------------
    d_ids = nc.dram_tensor("ids", [M], i32, kind="ExternalInput")
    d_maskb = nc.dram_tensor("maskb", [P, SEQ * DC], f32, kind="ExternalInput")
    d_tok = nc.dram_tensor("tok_emb", [V, D], f32, kind="ExternalInput")
    d_posT = nc.dram_tensor("posT", [D, S], f16, kind="ExternalInput")
    d_wqkv = nc.dram_tensor("wqkv", [L, 3, D, D], f16, kind="ExternalInput")
    d_wo = nc.dram_tensor("wo", [L, D, D], f16, kind="ExternalInput")
    d_bq = nc.dram_tensor("bq", [L, D], f32, kind="ExternalInput")
    d_bk = nc.dram_tensor("bk", [L, D], f32, kind="ExternalInput")
    d_bv = nc.dram_tensor("bv", [L, D], f32, kind="ExternalInput")
    d_bo = nc.dram_tensor("bo", [L, D], f32, kind="ExternalInput")
    d_ln1s = nc.dram_tensor("ln1s", [L, D], f32, kind="ExternalInput")
    d_ln1b = nc.dram_tensor("ln1b", [L, D], f32, kind="ExternalInput")
    d_ln2s = nc.dram_tensor("ln2s", [L, D], f32, kind="ExternalInput")
    d_ln2b = nc.dram_tensor("ln2b", [L, D], f32, kind="ExternalInput")
    d_w1 = nc.dram_tensor("w1", [L, D, FF], f16, kind="ExternalInput")
    d_b1 = nc.dram_tensor("b1", [L, FF], f32, kind="ExternalInput")
    d_w2 = nc.dram_tensor("w2", [L, FF, D], f16, kind="ExternalInput")
    d_b2 = nc.dram_tensor("b2", [L, D], f32, kind="ExternalInput")
    d_lnfs = nc.dram_tensor("lnfs", [D], f32, kind="ExternalInput")
    d_lnfb = nc.dram_tensor("lnfb", [D], f32, kind="ExternalInput")
    d_c1w = nc.dram_tensor("c1w", [D, D // 2], f32, kind="ExternalInput")
    d_c1b = nc.dram_tensor("c1b", [D // 2], f32, kind="ExternalInput")
    d_c2w = nc.dram_tensor("c2w", [D // 2, D // 4], f32, kind="ExternalInput")
    d_c2b = nc.dram_tensor("c2b", [D // 4], f32, kind="ExternalInput")
    d_c3w = nc.dram_tensor("c3w", [D // 4], f32, kind="ExternalInput")
    d_c3b = nc.dram_tensor("c3b", [1], f32, kind="ExternalInput")
    d_out = nc.dram_tensor("logits", [SEQ], f32, kind="ExternalOutput")

    def fmaj(ap):
        # [(c p), n] DRAM -> [p, c, n] feature-major view
        return ap.rearrange("(c p) n -> p c n", p=P)

    def fvec(ap):
        # [(c p)] DRAM -> [p, c]
        return ap.rearrange("(c p) -> p c", p=P)

    with tile.TileContext(nc) as tc:
        with (
            tc.tile_pool(name="sb", bufs=1) as sb,
            tc.tile_pool(name="ps", bufs=1, space="PSUM") as ps,
        ):
            # ---------------- constants ----------------
            ones16 = sb.tile([P, 1], f16, tag="ones16")
            nc.vector.memset(ones16[:], 1.0)
            onesr_f = sb.tile([1, P], f32, tag="onesr_f")
            nc.vector.memset(onesr_f[:], 1.0)
            ones_row = sb.tile([1, P], f32r, tag="ones_row")
            nc.vector.tensor_copy(ones_row[:], onesr_f[:])
            eps_t = sb.tile([1, 1], f32, tag="eps")
            nc.vector.memset(eps_t[:], EPS)
            maskb_t = sb.tile([P, SEQ * DC], f32, tag="maskb")
            nc.sync.dma_start(maskb_t[:], d_maskb[:])

            # ---------------- embedding ----------------
            # tiles borrow tags whose first real use comes after embedding
            ident = sb.tile([P, P], f32, tag="recbc", bufs=1, name="ident")
            make_identity(nc, ident[:])
            posT_t = sb.tile([P, DC, S], f16, tag="lnt", bufs=2, name="posT")
            nc.sync.dma_start(posT_t[:], fmaj(d_posT[:]))
            xcur = []
            for s in range(SEQ):
                xt = sb.tile([P, DC, S], f16, tag="x", bufs=5, name="x")
                xcur.append(xt)
            idx_all = sb.tile([P, SEQ * DC], i32, tag="dens", bufs=2,
                              name="idx")
            nc.sync.dma_start(idx_all[:],
                              d_ids.rearrange("(t p) -> p t", p=P))
            for t in range(SEQ * DC):  # token tile: seq t//4, tok chunk t%4
                s, tc_ = t // DC, t % DC
                g = sb.tile([P, D], f32, tag="xs", bufs=2, name="g")
                nc.gpsimd.indirect_dma_start(
                    out=g[:], out_offset=None, in_=d_tok[:],
                    in_offset=bass.IndirectOffsetOnAxis(
                        ap=idx_all[:, t:t + 1], axis=0),
                )
                for fc in range(DC):
                    ptr = ps.tile([P, S], f32, space="PSUM", tag="pa",
                                  bufs=4, name="ptr")
                    nc.tensor.transpose(out=ptr[:, 0:P],
                                        in_=g[:, fc * P:(fc + 1) * P],
                                        identity=ident[:])
                    with nc.allow_low_precision(reason="f16 x"):
                        nc.vector.tensor_tensor(
                            out=xcur[s][:, fc, tc_ * P:(tc_ + 1) * P],
                            in0=ptr[:, 0:P],
                            in1=posT_t[:, fc, tc_ * P:(tc_ + 1) * P],
                            op=OP.add)

            # ---------------- layer-norm emitters (split for pipelining) ----
            def ln_reduce(xin, ptag="pr", pbufs=1):
                """sq + reduction matmuls -> packed stats PSUM [33,S]
                (row 0 = sum x, row 32 = sum x^2)."""
                sq = sb.tile([P, DC, S], f16, tag="sq", bufs=1, name="sq")
                with nc.allow_low_precision(reason="f16 squares; var tol ok"):
                    nc.vector.tensor_tensor(out=sq[:], in0=xin[:], in1=xin[:],
                                            op=OP.mult)
                pst = ps.tile([33, S], f32, space="PSUM", tag=ptag, bufs=pbufs,
                              name="pst")
                for kc in range(DC):
                    nc.tensor.matmul(pst[0:1, :], lhsT=ones16[:],
                                     rhs=xin[:, kc, :],
                                     start=(kc == 0), stop=(kc == DC - 1))
                for kc in range(DC):
                    nc.tensor.matmul(pst[32:33, :], lhsT=ones16[:],
                                     rhs=sq[:, kc, :],
                                     start=(kc == 0), stop=(kc == DC - 1))
                return pst

            def ln_stats(pst):
                """ACT/DVE chain: mean + rsqrt(var) rows ([1,S] f32r)."""
                m = sb.tile([1, S], f32r, tag="st", bufs=4, name="m")
                with nc.allow_low_precision(reason="f32r stats"):
                    nc.vector.tensor_scalar(out=m[:], in0=pst[0:1, :],
                                            scalar1=1.0 / D, scalar2=None,
                                            op0=OP.mult)
                mm = sb.tile([1, S], f32, tag="st", bufs=4, name="mm")
                with nc.allow_low_precision(reason="f32r stats"):
                    nc.vector.tensor_tensor(out=mm[:], in0=m[:], in1=m[:],
                                            op=OP.mult)
                var = sb.tile([1, S], f32, tag="st", bufs=4, name="var")
                nc.vector.scalar_tensor_tensor(
                    out=var[:], in0=pst[32:33, :], scalar=1.0 / D, in1=mm[:],
                    op0=OP.mult, op1=OP.subtract)
                lv = sb.tile([1, S], f32, tag="st", bufs=4, name="lv")
                nc.scalar.activation(out=lv[:], in_=var[:], func=AF.Ln,
                                     bias=eps_t[:], scale=1.0)
                r = sb.tile([1, S], f32r, tag="st", bufs=4, name="r")
                with nc.allow_low_precision(reason="f32r stats"):
                    nc.scalar.activation(out=r[:], in_=lv[:], func=AF.Exp,
                                         bias=0.0, scale=-0.5)
                return m, r

            def ln_apply(xin, m, r, xout, s_t, b_t):
                """broadcast m,r; normalize full tile; per-chunk affine."""
                bc_m = ps.tile([P, S], f32, space="PSUM", tag="pa", bufs=4,
                               name="bc_m")
                nc.tensor.matmul(bc_m[:], lhsT=ones_row[:], rhs=m[:],
                                 start=True, stop=True)
                bc_r = ps.tile([P, S], f32, space="PSUM", tag="pa", bufs=4,
                               name="bc_r")
                nc.tensor.matmul(bc_r[:], lhsT=ones_row[:], rhs=r[:],
                                 start=True, stop=True)
                # evacuate broadcasts to f16 SBUF: frees the pa banks quickly
                # and lets the big tensor_tensor ops run in 2x f16 mode
                bcs = sb.tile([P, 2, S], f16, tag="bcs", bufs=1, name="bcs")
                with nc.allow_low_precision(reason="f16 ln"):
                    nc.scalar.activation(out=bcs[:, 0, :], in_=bc_m[:],
                                         func=AF.Identity, bias=0.0, scale=1.0)
                    nc.scalar.activation(out=bcs[:, 1, :], in_=bc_r[:],
                                         func=AF.Identity, bias=0.0, scale=1.0)
                t1 = sb.tile([P, DC, S], f16, tag="lnt", bufs=2, name="t1")
                with nc.allow_low_precision(reason="f16 ln"):
                    nc.vector.tensor_tensor(
                        out=t1[:], in0=xin[:],
                        in1=bcs[:, 0, None, :].to_broadcast([P, DC, S]),
                        op=OP.subtract)
                    u = sb.tile([P, DC, S], f16, tag="lnt", bufs=2, name="u")
                    nc.vector.tensor_tensor(
                        out=u[:], in0=t1[:],
                        in1=bcs[:, 1, None, :].to_broadcast([P, DC, S]),
                        op=OP.mult)
                    for kc in range(DC):
                        nc.scalar.activation(out=xout[:, kc, :],
                                             in_=u[:, kc, :],
                                             func=AF.Identity,
                                             bias=b_t[:, kc:kc + 1],
                                             scale=s_t[:, kc:kc + 1])

            def emit_ln(xin, xout, s_t, b_t):
                pst = ln_reduce(xin)
                m, r = ln_stats(pst)
                ln_apply(xin, m, r, xout, s_t, b_t)

            # head-pair broadcast selector: row 0 -> partitions 0-63,
            # row 32 -> partitions 64-127; other rows zero (nullify garbage)
            sel33f = sb.tile([33, P], f32, tag="sel33f")
            nc.vector.memset(sel33f[:], 0.0)
            nc.vector.memset(sel33f[0:1, 0:HD], 1.0)
            nc.vector.memset(sel33f[32:33, HD:P], 1.0)
            sel33 = sb.tile([33, P], f32r, tag="sel33")
            nc.vector.tensor_copy(sel33[:], sel33f[:])

            # ---------------- transformer layers ----------------
            pending_tail = [None]   # deferred L2ln(3) emitter from layer i-1
            for i in range(n_layers):
                wqkv_t = sb.tile([P, 3, DC, D], f16, tag="wqkv", name="wqkv_t")
                nc.sync.dma_start(
                    wqkv_t[:],
                    d_wqkv[i].rearrange("j (c p) n -> p j c n", p=P))
                wo_t = sb.tile([P, DC, D], f16, tag="wo", name="wo_t")
                nc.sync.dma_start(wo_t[:], fmaj(d_wo[i]))
                w1_t = sb.tile([P, DC, FF], f16, tag="w1", name="w1_t")
                nc.sync.dma_start(w1_t[:], fmaj(d_w1[i]))
                w2_t = sb.tile([P, FFC, D], f16, tag="w2", name="w2_t")
                nc.sync.dma_start(w2_t[:], fmaj(d_w2[i]))
                bq_t = sb.tile([P, DC], f32, tag="bq", bufs=2, name="bq_t")
                bk_t = sb.tile([P, DC], f32, tag="bk", bufs=2, name="bk_t")
                bo_t = sb.tile([P, DC], f32, tag="bo", bufs=2, name="bo_t")
                b2_t = sb.tile([P, DC], f32, tag="b2", bufs=2, name="b2_t")
                b1_t = sb.tile([P, FFC], f32, tag="b1", bufs=2, name="b1_t")
                l1s_t = sb.tile([P, DC], f32, tag="l1s", bufs=2, name="l1s_t")
                l1b_t = sb.tile([P, DC], f32, tag="l1b", bufs=2, name="l1b_t")
                l2s_t = sb.tile([P, DC], f32, tag="l2s", bufs=2, name="l2s_t")
                l2b_t = sb.tile([P, DC], f32, tag="l2b", bufs=2, name="l2b_t")
                nc.sync.dma_start(bq_t[:], fvec(d_bq[i]))
                nc.sync.dma_start(bk_t[:], fvec(d_bk[i]))
                nc.sync.dma_start(bo_t[:], fvec(d_bo[i]))
                nc.sync.dma_start(b2_t[:], fvec(d_b2[i]))
                nc.sync.dma_start(b1_t[:], fvec(d_b1[i]))
                nc.sync.dma_start(l1s_t[:], fvec(d_ln1s[i]))
                nc.sync.dma_start(l1b_t[:], fvec(d_ln1b[i]))
                nc.sync.dma_start(l2s_t[:], fvec(d_ln2s[i]))
                nc.sync.dma_start(l2b_t[:], fvec(d_ln2b[i]))
                # bv broadcast to [P, H, HD] (bias along the free dim of V)
                bv_row = sb.tile([1, D], f32r, tag="bvrow", bufs=2,
                                 name="bv_row")
                nc.sync.dma_start(bv_row[:], d_bv[i][None, :].bitcast(f32r))
                pbv = ps.tile([P, D], f32, space="PSUM", tag="pa", bufs=4,
                              name="pbv")
                nc.tensor.matmul(pbv[:], lhsT=ones_row[:], rhs=bv_row[:],
                                 start=True, stop=True)
                bv_bc = sb.tile([P, H, HD], f16, tag="bvbc", bufs=2,
                                name="bv_bc")
                with nc.allow_low_precision(reason="f16 v bias"):
                    nc.vector.tensor_copy(
                        bv_bc[:], pbv[:].rearrange("p (h d) -> p h d", h=H))

                xnext = [None] * SEQ
                x2s = [None] * SEQ
                st_qkv = [None] * SEQ      # (qT, kT, vaug)
                st_att = [None] * SEQ      # (attnT, dens8, po_list)
                st_ln1 = [None] * SEQ      # (xsum, pst)
                st_ln2 = [None] * SEQ      # (xsum2, pst2)

                def stage_A(s):
                    x = xcur[s]
                    qT = sb.tile([P, DC, S], f16, tag="qT", bufs=2, name="qT")
                    kT = sb.tile([P, DC, S], f16, tag="kT", bufs=2, name="kT")
                    for dst, widx, bt in ((qT, 0, bq_t), (kT, 1, bk_t)):
                        for nck in range(DC):
                            pq = ps.tile([P, S], f32, space="PSUM", tag="pa",
                                         bufs=4, name="pq")
                            for kc in range(DC):
                                nc.tensor.matmul(
                                    pq[:],
                                    lhsT=wqkv_t[:, widx, kc,
                                                nck * P:(nck + 1) * P],
                                    rhs=x[:, kc, :],
                                    start=(kc == 0), stop=(kc == DC - 1))
                            with nc.allow_low_precision(reason="f16 qk"):
                                nc.vector.tensor_scalar(
                                    out=dst[:, nck, :], in0=pq[:],
                                    scalar1=bt[:, nck:nck + 1], scalar2=None,
                                    op0=OP.add)
                    vaug = sb.tile([P, DC, H, HD + 1], f16, tag="vaug",
                                   bufs=2, name="vaug")
                    for tc_ in range(DC):
                        pv = ps.tile([P, S], f32, space="PSUM", tag="pa",
                                     bufs=4, name="pv")
                        for kc in range(DC):
                            nc.tensor.matmul(
                                pv[:],
                                lhsT=x[:, kc, tc_ * P:(tc_ + 1) * P],
                                rhs=wqkv_t[:, 2, kc, :],
                                start=(kc == 0), stop=(kc == DC - 1))
                        with nc.allow_low_precision(reason="f16 v"):
                            nc.vector.tensor_tensor(
                                out=vaug[:, tc_, :, 0:HD],
                                in0=pv[:].rearrange("p (h d) -> p h d", h=H),
                                in1=bv_bc[:], op=OP.add)
                            nc.vector.tensor_copy(
                                out=vaug[:, tc_, :, HD:HD + 1],
                                in_=ones16[:, 0:1].to_broadcast([P, H, 1]))
                    st_qkv[s] = (qT, kT, vaug)

                def stage_B(s):
                    """scores/exp/PV per head pair, epilogue pipelined one
                    pair behind so the ACT ln/exp+bc never stall the PE."""
                    qT, kT, vaug = st_qkv[s]
                    attnT = sb.tile([P, DC, S], f16, tag="attnT", bufs=2,
                                    name="attnT")
                    po_pend = {}

                    def ep1a(hc):
                        # 1/den = exp(-ln(den)) on ACT (same table set as
                        # the softmax exp)
                        po0, po1, dens, _ = po_pend[hc]
                        lnd = sb.tile([33, S], f32, tag="rc2", bufs=2,
                                      name="lnd")
                        nc.scalar.activation(out=lnd[:], in_=dens[:],
                                             func=AF.Ln, bias=0.0, scale=1.0)
                        rec2 = sb.tile([33, S], f32r, tag="rc2", bufs=2,
                                       name="rec2")
                        with nc.allow_low_precision(reason="f32r rec"):
                            nc.scalar.activation(out=rec2[:], in_=lnd[:],
                                                 func=AF.Exp, bias=0.0,
                                                 scale=-1.0)
                        po_pend[hc] = (po0, po1, dens, rec2)

                    def ep1b(hc):
                        # broadcast the pair via the sel33 matmul
                        po0, po1, dens, rec2 = po_pend[hc]
                        pbc = ps.tile([P, S], f32, space="PSUM", tag="pbc",
                                      bufs=1, name="pbc")
                        nc.tensor.matmul(pbc[:], lhsT=sel33[:], rhs=rec2[:],
                                         start=True, stop=True)
                        recbc = sb.tile([P, S], f16, tag="recbc", bufs=1,
                                        name="recbc")
                        with nc.allow_low_precision(reason="f16 rec"):
                            nc.vector.tensor_copy(recbc[:], pbc[:])
                        po_pend[hc] = (po0, po1, dens, recbc)

                    def ep2(hc):
                        po0, po1, _, recbc = po_pend.pop(hc)
                        for hh, po in ((0, po0), (1, po1)):
                            hp = hh * HD
                            with nc.allow_low_precision(reason="f16 attn"):
                                nc.vector.scalar_tensor_tensor(
                                    out=attnT[hp:hp + HD, hc, :],
                                    in0=po[0:HD, :], scalar=1.0,
                                    in1=recbc[hp:hp + HD, :],
                                    op0=OP.mult, op1=OP.mult)

                    def scores(hc):
                        PTs = [sb.tile([P, DC, S], f16, tag="PT", bufs=4,
                                       name="PT") for _ in range(2)]
                        for kc in range(DC):
                            for hh in range(2):
                                hp = hh * HD
                                pS = ps.tile([P, S], f32, space="PSUM",
                                             tag="pa", bufs=4, name="pS")
                                nc.tensor.matmul(
                                    pS[:],
                                    lhsT=kT[hp:hp + HD, hc,
                                            kc * P:(kc + 1) * P],
                                    rhs=qT[hp:hp + HD, hc, :],
                                    start=True, stop=True)
                                with nc.allow_low_precision(reason="f16 attn"):
                                    nc.scalar.activation(
                                        out=PTs[hh][:, kc, :], in_=pS[:],
                                        func=AF.Exp,
                                        bias=maskb_t[:, s * DC + kc:
                                                     s * DC + kc + 1],
                                        scale=SCALE)
                        return PTs

                    def pv(hc, PTs):
                        dens = sb.tile([33, S], f32, tag="dens", bufs=2,
                                       name="dens")
                        nc.vector.memset(dens[:], 1.0)
                        pos = []
                        for hh in range(2):
                            h = hc * 2 + hh
                            PT = PTs[hh]
                            po = ps.tile([HD + 1, S], f32, space="PSUM",
                                         tag="pacc", bufs=2, name="po")
                            for kc in range(DC):
                                nc.tensor.matmul(po[:], lhsT=vaug[:, kc, h, :],
                                                 rhs=PT[:, kc, :],
                                                 start=(kc == 0),
                                                 stop=(kc == DC - 1))
                            nc.vector.tensor_copy(dens[32 * hh:32 * hh + 1, :],
                                                  po[HD:HD + 1, :])
                            pos.append(po)
                        po_pend[hc] = (pos[0], pos[1], dens, None)

                    # scores run one head-pair ahead of PV; the rec/broadcast
                    # epilogue trails one more pair behind. The epilogue is
                    # ordered (ACT ln/exp) -> scores MMs -> (bc MM, recbc,
                    # STTs) -> PV so no PE instruction waits on a long ACT
                    # queue and the pacc slot release precedes its reuse.
                    PT_pend = {}
                    for hc in range(H // 2):
                        if hc >= 2:
                            ep1a(hc - 2)
                        PT_pend[hc] = scores(hc)
                        if hc >= 2:
                            ep1b(hc - 2)
                            ep2(hc - 2)
                        if hc >= 1:
                            pv(hc - 1, PT_pend.pop(hc - 1))
                    for hc in (H // 2 - 2, H // 2 - 1):
                        ep1a(hc)
                        ep1b(hc)
                        ep2(hc)
                        if hc == H // 2 - 2:
                            pv(H // 2 - 1, PT_pend.pop(H // 2 - 1))
                    st_att[s] = attnT

                def stage_Cmm(s):
                    x = xcur[s]
                    attnT = st_att[s]
                    xsum = sb.tile([P, DC, S], f16, tag="xs", bufs=2,
                                   name="xsum")
                    for nck in range(DC):
                        pO = ps.tile([P, S], f32, space="PSUM", tag="pa",
                                     bufs=4, name="pO")
                        for kc in range(DC):
                            nc.tensor.matmul(
                                pO[:],
                                lhsT=wo_t[:, kc, nck * P:(nck + 1) * P],
                                rhs=attnT[:, kc, :],
                                start=(kc == 0), stop=(kc == DC - 1))
                        with nc.allow_low_precision(reason="f16 resid"):
                            nc.vector.scalar_tensor_tensor(
                                out=xsum[:, nck, :], in0=pO[:],
                                scalar=bo_t[:, nck:nck + 1],
                                in1=x[:, nck, :], op0=OP.add, op1=OP.add)
                    st_ln1[s] = (xsum, ln_reduce(xsum))

                def stage_Cln(s):
                    xsum, pst = st_ln1[s]
                    m, r = ln_stats(pst)
                    x2 = sb.tile([P, DC, S], f16, tag="x2", bufs=4, name="x2")
                    ln_apply(xsum, m, r, x2, l1s_t, l1b_t)
                    x2s[s] = x2

                def stage_D1(s):
                    x2 = x2s[s]
                    mid = sb.tile([P, FFC, S], f16, tag="mid", bufs=1,
                                  name="mid")
                    for fc in range(FFC):
                        pm = ps.tile([P, S], f32, space="PSUM", tag="pa",
                                     bufs=4, name="pm")
                        for kc in range(DC):
                            nc.tensor.matmul(
                                pm[:],
                                lhsT=w1_t[:, kc, fc * P:(fc + 1) * P],
                                rhs=x2[:, kc, :],
                                start=(kc == 0), stop=(kc == DC - 1))
                        with nc.allow_low_precision(reason="f16 mid"):
                            nc.scalar.activation(out=mid[:, fc, :], in_=pm[:],
                                                 func=GELU,
                                                 bias=b1_t[:, fc:fc + 1],
                                                 scale=1.0)
                    return mid

                def stage_D2(s, mid):
                    x2 = x2s[s]
                    xsum2 = sb.tile([P, DC, S], f16, tag="xs", bufs=2,
                                    name="xsum2")
                    for nck in range(DC):
                        pout = ps.tile([P, S], f32, space="PSUM", tag="pacc",
                                       bufs=2, name="pout")
                        for fc in range(FFC):
                            nc.tensor.matmul(
                                pout[:],
                                lhsT=w2_t[:, fc, nck * P:(nck + 1) * P],
                                rhs=mid[:, fc, :],
                                start=(fc == 0), stop=(fc == FFC - 1))
                        with nc.allow_low_precision(reason="f16 resid"):
                            nc.vector.scalar_tensor_tensor(
                                out=xsum2[:, nck, :], in0=pout[:],
                                scalar=b2_t[:, nck:nck + 1],
                                in1=x2[:, nck, :], op0=OP.add, op1=OP.add)
                    st_ln2[s] = (xsum2, ln_reduce(xsum2))

                def l2ln_impl(entry, xlist, s, ls_t, lb_t):
                    xsum2, pst2 = entry
                    m, r = ln_stats(pst2)
                    x3 = sb.tile([P, DC, S], f16, tag="x", bufs=5, name="x")
                    ln_apply(xsum2, m, r, x3, ls_t, lb_t)
                    xlist[s] = x3

                def stage_L2ln(s):
                    l2ln_impl(st_ln2[s], xnext, s, l2s_t, l2b_t)

                # pipelined emission: every serial ACT/DVE chain is covered
                # by an independent matmul block emitted between its producer
                # and its PE consumers.
                stage_A(0)
                if pending_tail[0] is not None:
                    pending_tail[0]()
                    pending_tail[0] = None
                stage_B(0)
                for s in range(SEQ):
                    if s + 1 < SEQ:
                        stage_A(s + 1)
                    stage_Cmm(s)
                    if s + 1 < SEQ:
                        stage_B(s + 1)
                        stage_Cln(s)
                mid0 = stage_D1(0)
                stage_Cln(SEQ - 1)
                stage_D2(0, mid0)
                for s in range(1, SEQ):
                    mids = stage_D1(s)
                    stage_L2ln(s - 1)
                    stage_D2(s, mids)
                pending_tail[0] = (
                    lambda f=l2ln_impl, e=st_ln2[SEQ - 1], xl=xnext,
                           ls=l2s_t, lb=l2b_t: f(e, xl, SEQ - 1, ls, lb))
                xcur = xnext

            # ---------------- final LN + classifier ----------------
            lfs_t = sb.tile([P, DC], f32, tag="l1s", bufs=2, name="lfs_t")
            lfb_t = sb.tile([P, DC], f32, tag="l1b", bufs=2, name="lfb_t")
            nc.sync.dma_start(lfs_t[:], fvec(d_lnfs[:]))
            nc.sync.dma_start(lfb_t[:], fvec(d_lnfb[:]))
            c1w_t = sb.tile([P, DC, D // 2], f32r, tag="c1w", name="c1w_t")
            nc.sync.dma_start(c1w_t[:], fmaj(d_c1w[:]).bitcast(f32r))
            c2w_t = sb.tile([P, 2, D // 4], f32r, tag="c2w", name="c2w_t")
            nc.sync.dma_start(c2w_t[:], fmaj(d_c2w[:]).bitcast(f32r))
            c3w_t = sb.tile([P, 1], f32r, tag="c3w", name="c3w_t")
            nc.sync.dma_start(c3w_t[:], d_c3w[:, None].bitcast(f32r))
            c1b_t = sb.tile([P, 2], f32, tag="c1b", name="c1b_t")
            nc.sync.dma_start(c1b_t[:], fvec(d_c1b[:]))
            c2b_t = sb.tile([P, 1], f32, tag="c2b", name="c2b_t")
            nc.sync.dma_start(c2b_t[:], fvec(d_c2b[:]))
            c3b_t = sb.tile([1, 1], f32, tag="c3b", name="c3b_t")
            nc.sync.dma_start(c3b_t[:], d_c3b[None, :])

            if pending_tail[0] is not None:
                pending_tail[0]()
                pending_tail[0] = None
            # final LN is only needed for the CLS token (column 0) of each
            # sequence: gather the 4 columns, then run a micro-LN on [P,DC,4]
            clsR = sb.tile([P, DC, SEQ], f16, tag="clsR", name="clsR")
            for s in range(SEQ):
                nc.vector.tensor_copy(out=clsR[:, :, s:s + 1],
                                      in_=xcur[s][:, :, 0:1])
            sqc = sb.tile([P, DC, SEQ], f16, tag="sqc", name="sqc")
            with nc.allow_low_precision(reason="f16 cls"):
                nc.vector.tensor_tensor(out=sqc[:], in0=clsR[:], in1=clsR[:],
                                        op=OP.mult)
            pstc = ps.tile([33, SEQ], f32, space="PSUM", tag="pa", bufs=4,
                           name="pstc")
            for kc in range(DC):
                nc.tensor.matmul(pstc[0:1, :], lhsT=ones16[:],
                                 rhs=clsR[:, kc, :],
                                 start=(kc == 0), stop=(kc == DC - 1))
            for kc in range(DC):
                nc.tensor.matmul(pstc[32:33, :], lhsT=ones16[:],
                                 rhs=sqc[:, kc, :],
                                 start=(kc == 0), stop=(kc == DC - 1))
            mc = sb.tile([1, SEQ], f32r, tag="stc", bufs=5, name="mc")
            with nc.allow_low_precision(reason="f32r stats"):
                nc.vector.tensor_scalar(out=mc[:], in0=pstc[0:1, :],
                                        scalar1=1.0 / D, scalar2=None,
                                        op0=OP.mult)
            mmc = sb.tile([1, SEQ], f32, tag="stc", bufs=5, name="mmc")
            with nc.allow_low_precision(reason="f32r stats"):
                nc.vector.tensor_tensor(out=mmc[:], in0=mc[:], in1=mc[:],
                                        op=OP.mult)
            varc = sb.tile([1, SEQ], f32, tag="stc", bufs=5, name="varc")
            nc.vector.scalar_tensor_tensor(
                out=varc[:], in0=pstc[32:33, :], scalar=1.0 / D, in1=mmc[:],
                op0=OP.mult, op1=OP.subtract)
            lvc = sb.tile([1, SEQ], f32, tag="stc", bufs=5, name="lvc")
            nc.scalar.activation(out=lvc[:], in_=varc[:], func=AF.Ln,
                                 bias=eps_t[:], scale=1.0)
            rc = sb.tile([1, SEQ], f32r, tag="stc", bufs=5, name="rc")
            with nc.allow_low_precision(reason="f32r stats"):
                nc.scalar.activation(out=rc[:], in_=lvc[:], func=AF.Exp,
                                     bias=0.0, scale=-0.5)
            bcm = ps.tile([P, SEQ], f32, space="PSUM", tag="pa", bufs=4,
                          name="bcm")
            nc.tensor.matmul(bcm[:], lhsT=ones_row[:], rhs=mc[:],
                             start=True, stop=True)
            bcr = ps.tile([P, SEQ], f32, space="PSUM", tag="pa", bufs=4,
                          name="bcr")
            nc.tensor.matmul(bcr[:], lhsT=ones_row[:], rhs=rc[:],
                             start=True, stop=True)
            clsT = sb.tile([P, DC, SEQ], f32r, tag="clsT", name="clsT")
            t1c = sb.tile([P, DC, SEQ], f32, tag="sqc2", name="t1c")
            with nc.allow_low_precision(reason="cls ln"):
                nc.vector.tensor_tensor(
                    out=t1c[:], in0=clsR[:],
                    in1=bcm[:, None, :].to_broadcast([P, DC, SEQ]),
                    op=OP.subtract)
                u1c = sb.tile([P, DC, SEQ], f32, tag="sqc3", name="u1c")
                nc.vector.tensor_tensor(
                    out=u1c[:], in0=t1c[:],
                    in1=bcr[:, None, :].to_broadcast([P, DC, SEQ]),
                    op=OP.mult)
                for kc in range(DC):
                    nc.scalar.activation(out=clsT[:, kc, :],
                                         in_=u1c[:, kc, :],
                                         func=AF.Identity,
                                         bias=lfb_t[:, kc:kc + 1],
                                         scale=lfs_t[:, kc:kc + 1])
            h1 = sb.tile([P, 2, SEQ], f32r, tag="h1", name="h1")
            for nn in range(2):
                pc = ps.tile([P, SEQ], f32, space="PSUM", tag="pr", bufs=1,
                             name="pc")
                for kc in range(DC):
                    nc.tensor.matmul(pc[:],
                                     lhsT=c1w_t[:, kc, nn * P:(nn + 1) * P],
                                     rhs=clsT[:, kc, :],
                                     start=(kc == 0), stop=(kc == DC - 1))
                with nc.allow_low_precision(reason="f32r cls"):
                    nc.scalar.activation(out=h1[:, nn, :], in_=pc[:],
                                         func=GELU,
                                         bias=c1b_t[:, nn:nn + 1], scale=1.0)
            h2 = sb.tile([P, SEQ], f32r, tag="h2", name="h2")
            pc2 = ps.tile([P, SEQ], f32, space="PSUM", tag="pr", bufs=1,
                          name="pc2")
            for cc in range(2):
                nc.tensor.matmul(pc2[:], lhsT=c2w_t[:, cc, :],
                                 rhs=h1[:, cc, :],
                                 start=(cc == 0), stop=(cc == 1))
            with nc.allow_low_precision(reason="f32r cls"):
                nc.scalar.activation(out=h2[:], in_=pc2[:], func=GELU,
                                     bias=c2b_t[:], scale=1.0)
            pc3 = ps.tile([1, SEQ], f32, space="PSUM", tag="pr", bufs=1,
                          name="pc3")
            nc.tensor.matmul(pc3[:], lhsT=c3w_t[:], rhs=h2[:],
                             start=True, stop=True)
            lg = sb.tile([1, SEQ], f32, tag="lg", name="lg")
            nc.scalar.activation(out=lg[:], in_=pc3[:], func=AF.Identity,
                                 bias=c3b_t[:], scale=1.0)
            nc.sync.dma_start(d_out[None, :], lg[:])

    nc.compile()
    return nc


def get_nc(n_layers=L):
    if n_layers not in _CACHE:
        _CACHE[n_layers] = _build(n_layers)
    return _CACHE[n_layers]


def make_in_maps(inputs):
    """Shard full inputs into per-core input maps (host-side prep)."""
    f = np.ascontiguousarray
    f16 = np.float16
    ids_all = np.asarray(inputs['input_ids']).astype(np.int32)        # [B,S]
    mask_all = np.asarray(inputs['attention_mask'])                   # [B,S]
    maskb_all = np.where(mask_all == 0, -1e9, 0.0).astype(np.float32)
    posT = f((np.asarray(inputs['pos_emb'])
              + np.asarray(inputs['type_emb'])[0][None, :]).T.astype(f16))
    wqkv = np.stack([np.asarray(inputs['Wq'], dtype=f16),
                     np.asarray(inputs['Wk'], dtype=f16),
                     np.asarray(inputs['Wv'], dtype=f16)], axis=1)  # [L,3,D,D]
    shared = dict(
        tok_emb=f(np.asarray(inputs['tok_emb'], dtype=np.float32)),
        posT=posT,
        wqkv=f(wqkv),
        wo=f(np.asarray(inputs['Wo'], dtype=f16)),
        bq=f(np.asarray(inputs['bq'], dtype=np.float32)),
        bk=f(np.asarray(inputs['bk'], dtype=np.float32)),
        bv=f(np.asarray(inputs['bv'], dtype=np.float32)),
        bo=f(np.asarray(inputs['bo'], dtype=np.float32)),
        ln1s=f(np.asarray(inputs['ln_a_s'], dtype=np.float32)),
        ln1b=f(np.asarray(inputs['ln_a_b'], dtype=np.float32)),
        ln2s=f(np.asarray(inputs['ln2_s'], dtype=np.float32)),
        ln2b=f(np.asarray(inputs['ln2_b'], dtype=np.float32)),
        w1=f(np.asarray(inputs['W1'], dtype=f16)),
        b1=f(np.asarray(inputs['b1'], dtype=np.float32)),
        w2=f(np.asarray(inputs['W2'], dtype=f16)),
        b2=f(np.asarray(inputs['b2'], dtype=np.float32)),
        lnfs=f(np.asarray(inputs['lnf_s'], dtype=np.float32)),
        lnfb=f(np.asarray(inputs['lnf_b'], dtype=np.float32)),
        c1w=f(np.asarray(inputs['c1W'], dtype=np.float32)),
        c1b=f(np.asarray(inputs['c1b'], dtype=np.float32)),
        c2w=f(np.asarray(inputs['c2W'], dtype=np.float32)),
        c2b=f(np.asarray(inputs['c2b'], dtype=np.float32)),
        c3w=f(np.asarray(inputs['c3W'], dtype=np.float32).reshape(D // 4)),
        c3b=f(np.asarray(inputs['c3b'], dtype=np.float32).reshape(1)),
    )
    in_maps = []
    for c in range(NCORES):
        ids_c = f(ids_all[c * SEQ:(c + 1) * SEQ].reshape(M))
        mb = maskb_all[c * SEQ:(c + 1) * SEQ]                 # [SEQ, S]
        # tile[p, s*DC+kc] = maskb[s, kc*128+p]
        mb_t = f(mb.reshape(SEQ, DC, P).transpose(2, 0, 1).reshape(P, SEQ * DC))
        in_maps.append(dict(shared, ids=ids_c, maskb=mb_t))
    return in_maps


def kernel(**inputs):
    from concourse.bass_utils import run_bass_kernel_spmd
    nc = get_nc(L)
    in_maps = make_in_maps(inputs)
    res = run_bass_kernel_spmd(nc, in_maps, list(range(NCORES)))
    out = np.concatenate([np.asarray(r["logits"], dtype=np.float32).reshape(SEQ)
                          for r in res.results])
    return out
